# revision 8
# baseline (speedup 1.0000x reference)
"""MHA (projections + masked softmax attention) on 8 NeuronCores.

Strategy: data-parallel over batch (B=8 -> 1 batch element per core, no
collectives). Per core, everything is computed in a transposed layout:

  QT = Wq^T @ x_q^T   [D, Sq]   (lhsT = Wq natural, rhs = x_q^T from host)
  KT = Wk^T @ x_k^T   [D, Sk]
  V  = x_v  @ Wv      [Sk, D]   (lhsT = x_v^T chunk, rhs = Wv natural)

Attention per head h in the "scores transposed" layout S^T[k, q]:
  S^T tile = KT_h_chunk.T @ QT_h          (k on partitions, q on free dim)
  masked scores: copy_predicated(-480) then exp(0.125*s) => exp(-60) ~= 0
  O^T[d,q] & Z[q] in ONE accumulating matmul: lhsT = [V_h | ones] (65 cols)
  final:  O = transpose(O^T) * (1/Z)  per 128-query block.

Host side: per-core transposes, queries sorted by valid_len (enables
column-suffix skipping of fully-masked key chunks and narrow predicated
ranges), mask precompute (bf16), and exact fixup of valid_len==0 rows
(reference gives uniform softmax there -> row = mean(value) @ Wv).
"""

import sys

if "/opt/trn_rl_repo" not in sys.path:
    sys.path.insert(0, "/opt/trn_rl_repo")

import numpy as np

B, S, D, H = 8, 1024, 1024, 16
DH = D // H  # 64
P = 128  # sbuf partitions
NQ = 512  # matmul free-dim tile (1 psum bank of fp32)
KC = S // P  # 8 key chunks
DC = D // P  # 8 hidden chunks
QH = S // NQ  # 2 query halves
N_CORES = 8
NEG = -480.0  # exp(0.125 * -480) = exp(-60) ~= 8.8e-27


def _build_nc(col_start, pred_end):
    """col_start[qh][kc]: first q column (within half qh, multiple of 128,
    0..512) that needs key-chunk kc; 512 = chunk fully skipped for the half.
    pred_end[qh][kc]: end (exclusive, multiple of 128) of the column range
    that needs mask predication; columns >= pred_end are fully valid.
    Both are unions over all cores. col_start[qh][0] must be 0."""
    from contextlib import ExitStack

    import concourse.mybir as mybir
    import concourse.tile as tile
    from concourse import bacc
    from concourse.masks import make_identity

    fp32 = mybir.dt.float32
    u8 = mybir.dt.uint8
    AF = mybir.ActivationFunctionType

    nc = bacc.Bacc(
        "TRN2",
        target_bir_lowering=False,
        debug=False,
        enable_asserts=False,
        num_devices=N_CORES,
    )

    xqT = nc.dram_tensor("xqT", (D, S), fp32, kind="ExternalInput").ap()
    xkT = nc.dram_tensor("xkT", (D, S), fp32, kind="ExternalInput").ap()
    xvT = nc.dram_tensor("xvT", (D, S), fp32, kind="ExternalInput").ap()
    wq = nc.dram_tensor("wq", (D, D), fp32, kind="ExternalInput").ap()
    wk = nc.dram_tensor("wk", (D, D), fp32, kind="ExternalInput").ap()
    wv = nc.dram_tensor("wv", (D, D), fp32, kind="ExternalInput").ap()
    maskT = nc.dram_tensor("maskT", (S, S), u8, kind="ExternalInput").ap()
    out = nc.dram_tensor("out", (S, D), fp32, kind="ExternalOutput").ap()

    with ExitStack() as ctx:
        tc = ctx.enter_context(tile.TileContext(nc))
        const = ctx.enter_context(tc.tile_pool(name="const", bufs=1))
        persist = ctx.enter_context(tc.tile_pool(name="persist", bufs=1))
        wpool = ctx.enter_context(tc.tile_pool(name="wpool", bufs=1))
        xpool = ctx.enter_context(tc.tile_pool(name="xpool", bufs=3))
        ppool = ctx.enter_context(tc.tile_pool(name="ppool", bufs=1, space="PSUM"))
        epool = ctx.enter_context(tc.tile_pool(name="epool", bufs=4))
        mpool = ctx.enter_context(tc.tile_pool(name="mpool", bufs=2))

        ident = const.tile([P, P], fp32)
        make_identity(nc, ident[:])
        negt = const.tile([P, NQ], fp32)
        nc.gpsimd.memset(negt[:], NEG)

        # persistent SBUF tensors
        qt_sb = [persist.tile([P, S], fp32, tag=f"qt{i}", name=f"qt{i}") for i in range(DC)]
        kt_sb = [persist.tile([P, S], fp32, tag=f"kt{i}", name=f"kt{i}") for i in range(DC)]
        va_sb = [persist.tile([P, H * (DH + 1)], fp32, tag=f"va{i}", name=f"va{i}") for i in range(KC)]
        mk_sb = [persist.tile([P, S], u8, tag=f"mk{i}", name=f"mk{i}") for i in range(KC)]
        for kc in range(KC):
            nc.sync.dma_start(mk_sb[kc][:], maskT[kc * P : (kc + 1) * P, :])
            # ones columns of V_aug (col DH of each 65-wide head block)
            va3 = va_sb[kc].rearrange("p (h d) -> p h d", d=DH + 1)
            nc.vector.memset(va3[:, :, DH], 1.0)

        # ---- Q/K projections: out[d, q] accumulating over hidden-in chunks
        def project_t(w_dram, x_dram, dst_sb):
            w_sb = [wpool.tile([P, D], fp32, tag=f"w{i}", name=f"w{i}") for i in range(DC)]
            for dc in range(DC):
                nc.sync.dma_start(w_sb[dc][:], w_dram[dc * P : (dc + 1) * P, :])
            for qh in range(QH):
                acc = [ppool.tile([P, NQ], fp32, tag=f"b{i}", name=f"acc{i}") for i in range(DC)]
                for dc in range(DC):
                    xt = xpool.tile([P, NQ], fp32, tag="xt")
                    nc.sync.dma_start(
                        xt[:], x_dram[dc * P : (dc + 1) * P, qh * NQ : (qh + 1) * NQ]
                    )
                    for oc in range(DC):
                        nc.tensor.matmul(
                            acc[oc][:],
                            w_sb[dc][:, oc * P : (oc + 1) * P],
                            xt[:],
                            start=(dc == 0),
                            stop=(dc == DC - 1),
                        )
                for oc in range(DC):
                    nc.scalar.copy(
                        dst_sb[oc][:, qh * NQ : (qh + 1) * NQ], acc[oc][:]
                    )

        project_t(wq, xqT, qt_sb)
        project_t(wk, xkT, kt_sb)

        # ---- V projection: out[k, d] tiles; contraction over hidden-in (dc)
        wv_sb = [wpool.tile([P, D], fp32, tag=f"w{i}", name=f"w{i}") for i in range(DC)]
        for dc in range(DC):
            nc.sync.dma_start(wv_sb[dc][:], wv[dc * P : (dc + 1) * P, :])
        for kc in range(KC):
            xvt = xpool.tile([P, DC, P], fp32, tag="xvt")  # [hid-part, dc, k]
            for dc in range(DC):
                nc.sync.dma_start(
                    xvt[:, dc, :], xvT[dc * P : (dc + 1) * P, kc * P : (kc + 1) * P]
                )
            for dhh in range(2):  # two 512-wide halves of hidden-out
                acc = ppool.tile([P, NQ], fp32, tag=f"b{(kc * 2 + dhh) % 2}", name="vacc")
                for dc in range(DC):
                    nc.tensor.matmul(
                        acc[:],
                        xvt[:, dc, :],
                        wv_sb[dc][:, dhh * NQ : (dhh + 1) * NQ],
                        start=(dc == 0),
                        stop=(dc == DC - 1),
                    )
                # scatter 8 heads x 64 cols into stride-65 head blocks
                dst = va_sb[kc][:, dhh * 8 * (DH + 1) : (dhh + 1) * 8 * (DH + 1)].rearrange(
                    "p (h d) -> p h d", h=8, d=DH + 1
                )[:, :, 0:DH]
                nc.vector.tensor_copy(
                    dst, acc[:].rearrange("p (h d) -> p h d", h=8, d=DH)
                )

        # ---- attention ----
        sc_rr = [0]
        for qh in range(QH):
            kcs = [kc for kc in range(KC) if col_start[qh][kc] < NQ]
            osb = [wpool.tile([P, D], fp32, tag=f"w{s}", name=f"osb{s}") for s in range(4)]
            for h in range(H):
                oc, ro = h // 2, (h % 2) * DH
                att = ppool.tile([DH + 1, NQ], fp32, tag=f"b{5 + h % 2}", name="att")
                for i, kc in enumerate(kcs):
                    c0 = col_start[qh][kc]
                    cv = pred_end[qh][kc]
                    sc = ppool.tile([P, NQ], fp32, tag=f"b{sc_rr[0] % 5}", name="sc")
                    sc_rr[0] += 1
                    nc.tensor.matmul(
                        sc[:, c0:],
                        kt_sb[oc][ro : ro + DH, kc * P : (kc + 1) * P],
                        qt_sb[oc][ro : ro + DH, qh * NQ + c0 : (qh + 1) * NQ],
                        start=True,
                        stop=True,
                    )
                    if cv > c0:
                        nc.vector.copy_predicated(
                            sc[:, c0:cv],
                            mk_sb[kc][:, qh * NQ + c0 : qh * NQ + cv],
                            negt[:, : cv - c0],
                        )
                    e = epool.tile([P, NQ], fp32, tag="e")
                    nc.scalar.activation(e[:, c0:], sc[:, c0:], AF.Exp, scale=0.125)
                    nc.tensor.matmul(
                        att[:, c0:],
                        va_sb[kc][:, h * (DH + 1) : (h + 1) * (DH + 1)],
                        e[:, c0:],
                        start=(i == 0),
                        stop=(i == len(kcs) - 1),
                    )
                # att rows 0:64 = O^T (unnormalized), row 64 = Z
                asb = mpool.tile([DH + 1, NQ], fp32, tag="asb")
                nc.vector.tensor_copy(asb[:], att[:])
                tr = ppool.tile([P, 4 * (DH + 1)], fp32, tag="b7", name="tr")
                for s in range(4):
                    nc.tensor.transpose(
                        tr[:, s * (DH + 1) : (s + 1) * (DH + 1)],
                        asb[:, s * P : (s + 1) * P],
                        ident[: DH + 1, : DH + 1],
                    )
                rz = mpool.tile([P, 4], fp32, tag="rz")
                tr3 = tr.rearrange("p (s d) -> p s d", d=DH + 1)
                nc.vector.reciprocal(rz[:], tr3[:, :, DH])
                for s in range(4):
                    nc.vector.tensor_scalar_mul(
                        osb[s][:, h * DH : (h + 1) * DH],
                        tr3[:, s, 0:DH],
                        rz[:, s : s + 1],
                    )
            for s in range(4):
                nc.sync.dma_start(
                    out[(qh * 4 + s) * P : (qh * 4 + s + 1) * P, :], osb[s][:]
                )

    nc.compile()
    return nc


_NC_CACHE = {}


def _get_nc(col_start, pred_end):
    key = (
        tuple(tuple(r) for r in col_start),
        tuple(tuple(r) for r in pred_end),
    )
    if key not in _NC_CACHE:
        _NC_CACHE[key] = _build_nc(col_start, pred_end)
    return _NC_CACHE[key]


def kernel(query, key, value, valid_len, Wq, Wk, Wv):
    from concourse import bass_utils

    query = np.asarray(query, dtype=np.float32)
    key = np.asarray(key, dtype=np.float32)
    value = np.asarray(value, dtype=np.float32)
    valid_len = np.asarray(valid_len, dtype=np.int32)
    Wq = np.asarray(Wq, dtype=np.float32)
    Wk = np.asarray(Wk, dtype=np.float32)
    Wv = np.asarray(Wv, dtype=np.float32)

    kidx = np.arange(S, dtype=np.int32)
    orders = []
    in_maps = []
    col_start = [[NQ] * KC for _ in range(QH)]
    pred_end = [[0] * KC for _ in range(QH)]
    for b in range(B):
        vl = valid_len[b]
        vl2 = np.where(vl == 0, 1, vl).astype(np.int32)
        order = np.argsort(vl2, kind="stable")
        orders.append(order)
        vs = vl2[order]
        for qh in range(QH):
            half = vs[qh * NQ : (qh + 1) * NQ]
            for kc in range(KC):
                need = half > (kc * P)  # chunk kc has >=1 valid key for col
                c0 = NQ if not need.any() else (int(np.argmax(need)) // P) * P
                col_start[qh][kc] = min(col_start[qh][kc], c0)
                full = half >= ((kc + 1) * P)  # fully valid -> no predication
                cv = NQ if not full.any() else int(np.argmax(full))
                cv = min(NQ, -(-cv // P) * P)
                pred_end[qh][kc] = max(pred_end[qh][kc], cv)
        xq_s = query[b][order]  # sorted queries
        maskT = (kidx[:, None] >= vs[None, :]).astype(np.uint8)
        in_maps.append(
            {
                "xqT": np.ascontiguousarray(xq_s.T),
                "xkT": np.ascontiguousarray(key[b].T),
                "xvT": np.ascontiguousarray(value[b].T),
                "wq": Wq,
                "wk": Wk,
                "wv": Wv,
                "maskT": maskT,
            }
        )

    nc = _get_nc(col_start, pred_end)
    global _LAST_IN_MAPS
    _LAST_IN_MAPS = in_maps
    res = bass_utils.run_bass_kernel_spmd(nc, in_maps, core_ids=list(range(N_CORES)))

    outs = np.empty((B, S, D), dtype=np.float32)
    for b in range(B):
        o_sorted = res.results[b]["out"]
        inv = np.empty(S, dtype=np.int64)
        inv[orders[b]] = np.arange(S)
        outs[b] = o_sorted[inv]
        zrows = np.where(valid_len[b] == 0)[0]
        if len(zrows):
            outs[b][zrows] = value[b].mean(axis=0) @ Wv
    return outs
